# revision 19
# baseline (speedup 1.0000x reference)
"""Trainium2 Bass kernel for dynamic-absmax-int8-quantized 3x3 conv (LUT form).

Math (verified bit-exact vs the jax reference on CPU):
  - lut[a+128, b+128] == a*b exactly, so the per-tap LUT sum is an integer
    matmul: out = sx*sw * (qw @ im2col(qx)) + bias.
  - max|patches(x)| == max|x| (the centre tap covers all of x).
  - jnp.round == fp32 round-to-nearest-even == (v + 1.5*2**23) - 1.5*2**23.
  - qx*qw products (<=16384) and their 576-term sums (<2**24) are exact in
    fp32, so the fp32 PE matmul is exact.
  - clip(-128,127) is a no-op: |v/s| <= 127*(1+eps) < 127.5.

Sharding: data-parallel over batch B=8, one image per core. Every core loads
the full x (401KB) to compute the global absmax locally, which is far cheaper
than the ~10us all-reduce floor.

Per-core dataflow:
  x_full[128,784]  -> DVE absmax-reduce -> PE transpose -> DVE reduce -> PE
  broadcast-matmul -> DVE /127 -> sx on all partitions.  Same for the weight.
  x_img[128,196] (image duplicated on partitions 0-63 / 64-127) is divided,
  magic-rounded, and written into a zero-padded [128,16,16] tile where the
  upper 64 partitions hold the image shifted up one row; this makes each
  (kh=0,kh=1) tap pair a single K=128 matmul.  kh=2 taps are K=64 matmuls.
  qw is quantized the same way and transposed per-tap on the PE.
  Final: out = (acc * sx*sw) + bias on DVE, DMA out.
"""

import numpy as np

import concourse.bacc as bacc
import concourse.bass as bass
import concourse.mybir as mybir
from concourse import bass_utils
from concourse.masks import make_identity
from concourse.tile import TileContext

F32 = mybir.dt.float32
MAGIC = 12582912.0  # 1.5 * 2**23: fp32 RNE rounding constant for |v| < 2**22
C127 = float(np.float32(1.0) / np.float32(127.0))  # exact fp32 1/127
ALU = mybir.AluOpType
AX = mybir.AxisListType
ACT_COPY = mybir.ActivationFunctionType.Copy

N_CORES = 8
B, CIN, H, W, COUT = 8, 64, 14, 14, 64
HW = H * W  # 196
KDIM = CIN * 9  # 576
# Host-side column order of w_in: 64-wide (=Cin) blocks, one per tap.
# (kh=0,kh=1) pairs first (contiguous per kw for single K=128 transposes),
# then the three kh=2 taps.
TAP_ORDER = [(0, 0), (1, 0), (0, 1), (1, 1), (0, 2), (1, 2),
             (2, 0), (2, 1), (2, 2)]


def build_nc() -> bass.Bass:
    nc = bacc.Bacc("TRN2", target_bir_lowering=False, debug=False,
                   num_devices=N_CORES)

    x_full = nc.dram_tensor("x_full", [128, 784], F32, kind="ExternalInput").ap()
    x_img = nc.dram_tensor("x_img", [128, HW], F32, kind="ExternalInput").ap()
    w_in = nc.dram_tensor("w_in", [COUT, KDIM], F32, kind="ExternalInput").ap()
    b_in = nc.dram_tensor("b_in", [COUT, 1], F32, kind="ExternalInput").ap()
    out_d = nc.dram_tensor("out", [COUT, HW], F32, kind="ExternalOutput").ap()

    with TileContext(nc) as tc:
        with (
            tc.tile_pool(name="sb", bufs=1) as sb,
            tc.tile_pool(name="ps", bufs=1, space="PSUM") as ps,
        ):
            # ---- constants (no input deps; overlap the input DMAs) ----
            ident = sb.tile([128, 128], F32, tag="ident")
            make_identity(nc, ident)
            ones = sb.tile([1, 128], F32, tag="ones")
            nc.vector.memset(ones, 1.0)
            qxp = sb.tile([128, 16, 16], F32, tag="qxp")
            nc.vector.memset(qxp, 0.0)

            # ---- input DMAs (chunked for queue parallelism) ----
            xf = sb.tile([128, 784], F32, tag="xf")
            for j in range(4):
                nc.sync.dma_start(out=xf[:, 196 * j:196 * (j + 1)],
                                  in_=x_full[:, 196 * j:196 * (j + 1)])
            xi = sb.tile([128, HW], F32, tag="xi")
            nc.sync.dma_start(out=xi, in_=x_img)
            wt = sb.tile([COUT, KDIM], F32, tag="wt")
            for j in range(2):
                nc.sync.dma_start(out=wt[:, 288 * j:288 * (j + 1)],
                                  in_=w_in[:, 288 * j:288 * (j + 1)])
            bt = sb.tile([COUT, 1], F32, tag="bt")
            nc.sync.dma_start(out=bt, in_=b_in)

            # ---- global absmax of x -> sx broadcast to all partitions ----
            cmax4 = sb.tile([128, 4], F32, tag="cmax4")
            for j in range(4):
                nc.vector.tensor_reduce(
                    out=cmax4[:, j:j + 1], in_=xf[:, 196 * j:196 * (j + 1)],
                    axis=AX.X, op=ALU.max, apply_absolute_value=True)
            cmax = sb.tile([128, 1], F32, tag="cmax")
            nc.vector.tensor_reduce(out=cmax, in_=cmax4, axis=AX.X, op=ALU.max)
            ptx = ps.tile([1, 128], F32, tag="tps", bufs=3)
            nc.tensor.transpose(ptx, cmax, ident)
            mxx = sb.tile([1, 1], F32, tag="mxx")
            nc.vector.tensor_reduce(out=mxx, in_=ptx, axis=AX.X, op=ALU.max)
            pbx = ps.tile([128, 1], F32, tag="pbc", bufs=2)
            nc.tensor.matmul(pbx, ones, mxx, start=True, stop=True)
            # sx = absmax/127 via mult by the exact fp32 constant 1/127, then
            # the bit-exact iterative reciprocal (DVE has no divide op).
            bcx = sb.tile([128, 1], F32, tag="bcx")
            nc.vector.tensor_scalar(out=bcx, in0=pbx, scalar1=C127,
                                    scalar2=None, op0=ALU.mult)
            rcx = sb.tile([128, 1], F32, tag="rcx")
            nc.vector.reciprocal(rcx, bcx)

            # ---- weight absmax -> sw broadcast ----
            wmax2 = sb.tile([COUT, 2], F32, tag="wmax2")
            for j in range(2):
                nc.vector.tensor_reduce(
                    out=wmax2[:, j:j + 1], in_=wt[:, 288 * j:288 * (j + 1)],
                    axis=AX.X, op=ALU.max, apply_absolute_value=True)
            wmax = sb.tile([COUT, 1], F32, tag="wmax")
            nc.vector.tensor_reduce(out=wmax, in_=wmax2, axis=AX.X, op=ALU.max)
            ptw = ps.tile([1, 64], F32, tag="tps", bufs=3)
            nc.tensor.transpose(ptw, wmax, ident[0:64, 0:64])
            mxw = sb.tile([1, 1], F32, tag="mxw")
            nc.vector.tensor_reduce(out=mxw, in_=ptw, axis=AX.X, op=ALU.max)
            pbw = ps.tile([COUT, 1], F32, tag="pbc", bufs=2)
            nc.tensor.matmul(pbw, ones[0:1, 0:64], mxw, start=True, stop=True)
            bcw = sb.tile([COUT, 1], F32, tag="bcw")
            nc.vector.tensor_scalar(out=bcw, in0=pbw, scalar1=C127,
                                    scalar2=None, op0=ALU.mult)
            rcw = sb.tile([COUT, 1], F32, tag="rcw")
            nc.vector.reciprocal(rcw, bcw)

            # alpha = sx * sw (per-partition, 0..63)
            alpha = sb.tile([COUT, 1], F32, tag="alpha")
            nc.vector.tensor_mul(alpha, bcx[0:64, :], bcw)

            # ---- quantize x into the padded tile (both shifted copies) ----
            # qxp[p, r, c] = qx(h=r-1, w=c-1) on partitions 0-63 (== xpad),
            # qxp[p+64, r, c] = qx(h=r, w=c-1)  (shifted up one row).
            tq = sb.tile([128, H, W], F32, tag="tq")
            nc.vector.tensor_scalar(out=tq, in0=xi.rearrange("p (h w) -> p h w", w=W),
                                    scalar1=rcx, scalar2=MAGIC,
                                    op0=ALU.mult, op1=ALU.add)
            nc.scalar.activation(out=qxp[0:64, 1:15, 1:15], in_=tq[0:64],
                                 func=ACT_COPY, bias=-MAGIC)
            nc.scalar.activation(out=qxp[64:128, 0:14, 1:15], in_=tq[64:128],
                                 func=ACT_COPY, bias=-MAGIC)

            # ---- quantize w ----
            wtq = sb.tile([COUT, KDIM], F32, tag="wtq")
            nc.vector.tensor_scalar(out=wtq, in0=wt, scalar1=rcw, scalar2=MAGIC,
                                    op0=ALU.mult, op1=ALU.add)
            wq = sb.tile([COUT, KDIM], F32, tag="wq")
            nc.scalar.activation(out=wq, in_=wtq, func=ACT_COPY, bias=-MAGIC)

            # ---- per-tap weight transposes on the PE ----
            # w_in columns are host-permuted to blocks of 64 (=Cin) in
            # TAP_ORDER, so each transpose input is contiguous 1-D free
            # (the BIR verifier rejects multi-dim free APs on the
            # stationary operand).  Transposed tiles are [tap*64+c, o].
            lhsT_p = sb.tile([128, 3, 64], F32, tag="lhsT_p")
            for kw in range(3):
                pst = ps.tile([128, 64], F32, tag="tps", bufs=3, name=f"pst{kw}")
                nc.tensor.transpose(pst, wq[:, 128 * kw:128 * (kw + 1)],
                                    ident[0:64, 0:64])
                nc.scalar.copy(lhsT_p[:, kw, :], pst)
            # kh=2 taps: zero-pad lhsT to K=128 so every conv matmul has the
            # same (128, 64) tile config — mixing K=64/K=128 matmuls in one
            # PSUM accumulation group faults at runtime.
            s2a = sb.tile([128, 64], F32, tag="s2a")
            s2b = sb.tile([128, 64], F32, tag="s2b")
            s1t = sb.tile([128, 64], F32, tag="s1t")
            nc.vector.memset(s2a, 0.0)
            nc.vector.memset(s2b, 0.0)
            nc.vector.memset(s1t, 0.0)
            pst3 = ps.tile([128, 64], F32, tag="tps", bufs=3)
            nc.tensor.transpose(pst3, wq[:, 384:512], ident[0:64, 0:64])
            nc.scalar.copy(s2a[0:64, :], pst3[0:64, :])
            nc.vector.tensor_copy(s2b[64:128, :], pst3[64:128, :])
            pst4 = ps.tile([64, 64], F32, tag="tps", bufs=3)
            nc.tensor.transpose(pst4, wq[:, 512:576], ident[0:64, 0:64])
            nc.vector.tensor_copy(s1t[0:64, :], pst4)

            # ---- conv: 3 paired K=128 matmuls + 3 K=64 matmuls ----
            acc = ps.tile([COUT, H, W], F32, tag="acc")
            for kw in range(3):
                nc.tensor.matmul(acc, lhsT_p[:, kw, :],
                                 qxp[:, 0:14, kw:kw + 14],
                                 start=(kw == 0), stop=False)
            nc.tensor.matmul(acc, s2a, qxp[:, 2:16, 0:14],
                             start=False, stop=False)
            nc.tensor.matmul(acc, s2b, qxp[:, 1:15, 1:15],
                             start=False, stop=False)
            nc.tensor.matmul(acc, s1t, qxp[:, 2:16, 2:16],
                             start=False, stop=True)

            # ---- scale + bias, store ----
            osb = sb.tile([COUT, HW], F32, tag="osb")
            nc.vector.tensor_scalar(out=osb,
                                    in0=acc.rearrange("p h w -> p (h w)"),
                                    scalar1=alpha, scalar2=bt,
                                    op0=ALU.mult, op1=ALU.add)
            nc.sync.dma_start(out=out_d, in_=osb)

    nc.compile()
    return nc


_NC = None


def _get_nc():
    global _NC
    if _NC is None:
        _NC = build_nc()
    return _NC


def make_in_maps(x, weight, bias):
    x = np.ascontiguousarray(np.asarray(x, dtype=np.float32))
    w4 = np.asarray(weight, dtype=np.float32).reshape(COUT, CIN, 3, 3)
    w = np.ascontiguousarray(
        np.concatenate([w4[:, :, kh, kw] for kh, kw in TAP_ORDER], axis=1))
    b = np.ascontiguousarray(np.asarray(bias, dtype=np.float32).reshape(COUT, 1))
    xf = np.ascontiguousarray(x.reshape(128, 784))
    in_maps = []
    for c in range(N_CORES):
        img = x[c].reshape(64, HW)
        xi = np.ascontiguousarray(np.concatenate([img, img], axis=0))
        in_maps.append({"x_full": xf, "x_img": xi, "w_in": w, "b_in": b})
    return in_maps


def kernel(x, weight, bias, lut=None, gradient_lut=None, **_unused):
    nc = _get_nc()
    in_maps = make_in_maps(x, weight, bias)
    res = bass_utils.run_bass_kernel_spmd(nc, in_maps,
                                          core_ids=list(range(N_CORES)))
    out = np.stack([res.results[c]["out"] for c in range(N_CORES)], axis=0)
    return np.ascontiguousarray(out.reshape(B, COUT, H, W).astype(np.float32))


# revision 20
# speedup vs baseline: 1.0085x; 1.0085x over previous
"""Trainium2 Bass kernel for dynamic-absmax-int8-quantized 3x3 conv (LUT form).

Math (verified bit-exact vs the jax reference on CPU):
  - lut[a+128, b+128] == a*b exactly, so the per-tap LUT sum is an integer
    matmul: out = sx*sw * (qw @ im2col(qx)) + bias.
  - max|patches(x)| == max|x| (the centre tap covers all of x).
  - jnp.round == fp32 round-to-nearest-even == (v + 1.5*2**23) - 1.5*2**23.
  - qx*qw products (<=16384) and their 576-term sums (<2**24) are exact in
    fp32, so the fp32 PE matmul is exact.
  - clip(-128,127) is a no-op: |v/s| <= 127*(1+eps) < 127.5.

Sharding: data-parallel over batch B=8, one image per core. Every core loads
the full x (401KB) to compute the global absmax locally, which is far cheaper
than the ~10us all-reduce floor.

Per-core dataflow:
  x_full[128,784]  -> DVE absmax-reduce -> PE transpose -> DVE reduce -> PE
  broadcast-matmul -> DVE /127 -> sx on all partitions.  Same for the weight.
  x_img[128,196] (image duplicated on partitions 0-63 / 64-127) is divided,
  magic-rounded, and written into a zero-padded [128,16,16] tile where the
  upper 64 partitions hold the image shifted up one row; this makes each
  (kh=0,kh=1) tap pair a single K=128 matmul.  kh=2 taps are K=64 matmuls.
  qw is quantized the same way and transposed per-tap on the PE.
  Final: out = (acc * sx*sw) + bias on DVE, DMA out.
"""

import numpy as np

import concourse.bacc as bacc
import concourse.bass as bass
import concourse.mybir as mybir
from concourse import bass_utils
from concourse.masks import make_identity
from concourse.tile import TileContext

F32 = mybir.dt.float32
MAGIC = 12582912.0  # 1.5 * 2**23: fp32 RNE rounding constant for |v| < 2**22
C127 = float(np.float32(1.0) / np.float32(127.0))  # exact fp32 1/127
ALU = mybir.AluOpType
AX = mybir.AxisListType
ACT_COPY = mybir.ActivationFunctionType.Copy

N_CORES = 8
B, CIN, H, W, COUT = 8, 64, 14, 14, 64
HW = H * W  # 196
KDIM = CIN * 9  # 576
# Host-side column order of w_in: 64-wide (=Cin) blocks, one per tap.
# (kh=0,kh=1) pairs first (contiguous per kw for single K=128 transposes),
# then the three kh=2 taps.
TAP_ORDER = [(0, 0), (1, 0), (0, 1), (1, 1), (0, 2), (1, 2),
             (2, 0), (2, 1), (2, 2)]


def build_nc() -> bass.Bass:
    nc = bacc.Bacc("TRN2", target_bir_lowering=False, debug=False,
                   num_devices=N_CORES)

    x_full = nc.dram_tensor("x_full", [128, 784], F32, kind="ExternalInput").ap()
    x_img = nc.dram_tensor("x_img", [128, HW], F32, kind="ExternalInput").ap()
    w_in = nc.dram_tensor("w_in", [COUT, KDIM], F32, kind="ExternalInput").ap()
    b_in = nc.dram_tensor("b_in", [COUT, 1], F32, kind="ExternalInput").ap()
    out_d = nc.dram_tensor("out", [COUT, HW], F32, kind="ExternalOutput").ap()

    with TileContext(nc) as tc:
        with (
            tc.tile_pool(name="sb", bufs=1) as sb,
            tc.tile_pool(name="ps", bufs=1, space="PSUM") as ps,
        ):
            # ---- constants (no input deps; overlap the input DMAs) ----
            ident = sb.tile([128, 128], F32, tag="ident")
            make_identity(nc, ident)
            ones = sb.tile([1, 128], F32, tag="ones")
            nc.vector.memset(ones, 1.0)
            qxp = sb.tile([128, 16, 16], F32, tag="qxp")
            nc.vector.memset(qxp, 0.0)

            # ---- input DMAs (chunked for queue parallelism) ----
            xf = sb.tile([128, 784], F32, tag="xf")
            nc.sync.dma_start(out=xf, in_=x_full)
            xi = sb.tile([128, HW], F32, tag="xi")
            nc.sync.dma_start(out=xi, in_=x_img)
            wt = sb.tile([COUT, KDIM], F32, tag="wt")
            nc.sync.dma_start(out=wt, in_=w_in)
            bt = sb.tile([COUT, 1], F32, tag="bt")
            nc.sync.dma_start(out=bt, in_=b_in)

            # ---- global absmax of x -> sx broadcast to all partitions ----
            cmax = sb.tile([128, 1], F32, tag="cmax")
            nc.vector.tensor_reduce(out=cmax, in_=xf, axis=AX.X, op=ALU.max,
                                    apply_absolute_value=True)
            ptx = ps.tile([1, 128], F32, tag="tps", bufs=3)
            nc.tensor.transpose(ptx, cmax, ident)
            mxx = sb.tile([1, 1], F32, tag="mxx")
            nc.vector.tensor_reduce(out=mxx, in_=ptx, axis=AX.X, op=ALU.max)
            pbx = ps.tile([128, 1], F32, tag="pbc", bufs=2)
            nc.tensor.matmul(pbx, ones, mxx, start=True, stop=True)
            # sx = absmax/127 via mult by the exact fp32 constant 1/127, then
            # the bit-exact iterative reciprocal (DVE has no divide op).
            bcx = sb.tile([128, 1], F32, tag="bcx")
            nc.vector.tensor_scalar(out=bcx, in0=pbx, scalar1=C127,
                                    scalar2=None, op0=ALU.mult)
            rcx = sb.tile([128, 1], F32, tag="rcx")
            nc.vector.reciprocal(rcx, bcx)

            # ---- weight absmax -> sw broadcast ----
            wmax = sb.tile([COUT, 1], F32, tag="wmax")
            nc.vector.tensor_reduce(out=wmax, in_=wt, axis=AX.X, op=ALU.max,
                                    apply_absolute_value=True)
            ptw = ps.tile([1, 64], F32, tag="tps", bufs=3)
            nc.tensor.transpose(ptw, wmax, ident[0:64, 0:64])
            mxw = sb.tile([1, 1], F32, tag="mxw")
            nc.vector.tensor_reduce(out=mxw, in_=ptw, axis=AX.X, op=ALU.max)
            pbw = ps.tile([COUT, 1], F32, tag="pbc", bufs=2)
            nc.tensor.matmul(pbw, ones[0:1, 0:64], mxw, start=True, stop=True)
            bcw = sb.tile([COUT, 1], F32, tag="bcw")
            nc.vector.tensor_scalar(out=bcw, in0=pbw, scalar1=C127,
                                    scalar2=None, op0=ALU.mult)
            rcw = sb.tile([COUT, 1], F32, tag="rcw")
            nc.vector.reciprocal(rcw, bcw)

            # alpha = sx * sw (per-partition, 0..63)
            alpha = sb.tile([COUT, 1], F32, tag="alpha")
            nc.vector.tensor_mul(alpha, bcx[0:64, :], bcw)

            # ---- quantize x into the padded tile (both shifted copies) ----
            # qxp[p, r, c] = qx(h=r-1, w=c-1) on partitions 0-63 (== xpad),
            # qxp[p+64, r, c] = qx(h=r, w=c-1)  (shifted up one row).
            tq = sb.tile([128, H, W], F32, tag="tq")
            nc.vector.tensor_scalar(out=tq, in0=xi.rearrange("p (h w) -> p h w", w=W),
                                    scalar1=rcx, scalar2=MAGIC,
                                    op0=ALU.mult, op1=ALU.add)
            nc.scalar.activation(out=qxp[0:64, 1:15, 1:15], in_=tq[0:64],
                                 func=ACT_COPY, bias=-MAGIC)
            nc.scalar.activation(out=qxp[64:128, 0:14, 1:15], in_=tq[64:128],
                                 func=ACT_COPY, bias=-MAGIC)

            # ---- quantize w ----
            wtq = sb.tile([COUT, KDIM], F32, tag="wtq")
            nc.vector.tensor_scalar(out=wtq, in0=wt, scalar1=rcw, scalar2=MAGIC,
                                    op0=ALU.mult, op1=ALU.add)
            wq = sb.tile([COUT, KDIM], F32, tag="wq")
            nc.scalar.activation(out=wq, in_=wtq, func=ACT_COPY, bias=-MAGIC)

            # ---- per-tap weight transposes on the PE ----
            # w_in columns are host-permuted to blocks of 64 (=Cin) in
            # TAP_ORDER, so each transpose input is contiguous 1-D free
            # (the BIR verifier rejects multi-dim free APs on the
            # stationary operand).  Transposed tiles are [tap*64+c, o].
            lhsT_p = sb.tile([128, 3, 64], F32, tag="lhsT_p")
            for kw in range(3):
                pst = ps.tile([128, 64], F32, tag="tps", bufs=3, name=f"pst{kw}")
                nc.tensor.transpose(pst, wq[:, 128 * kw:128 * (kw + 1)],
                                    ident[0:64, 0:64])
                nc.scalar.copy(lhsT_p[:, kw, :], pst)
            # kh=2 taps: zero-pad lhsT to K=128 so every conv matmul has the
            # same (128, 64) tile config — mixing K=64/K=128 matmuls in one
            # PSUM accumulation group faults at runtime.
            s2a = sb.tile([128, 64], F32, tag="s2a")
            s2b = sb.tile([128, 64], F32, tag="s2b")
            s1t = sb.tile([128, 64], F32, tag="s1t")
            nc.vector.memset(s2a, 0.0)
            nc.vector.memset(s2b, 0.0)
            nc.vector.memset(s1t, 0.0)
            pst3 = ps.tile([128, 64], F32, tag="tps", bufs=3)
            nc.tensor.transpose(pst3, wq[:, 384:512], ident[0:64, 0:64])
            nc.scalar.copy(s2a[0:64, :], pst3[0:64, :])
            nc.vector.tensor_copy(s2b[64:128, :], pst3[64:128, :])
            pst4 = ps.tile([64, 64], F32, tag="tps", bufs=3)
            nc.tensor.transpose(pst4, wq[:, 512:576], ident[0:64, 0:64])
            nc.vector.tensor_copy(s1t[0:64, :], pst4)

            # ---- conv: 3 paired K=128 matmuls + 3 K=64 matmuls ----
            acc = ps.tile([COUT, H, W], F32, tag="acc")
            for kw in range(3):
                nc.tensor.matmul(acc, lhsT_p[:, kw, :],
                                 qxp[:, 0:14, kw:kw + 14],
                                 start=(kw == 0), stop=False)
            nc.tensor.matmul(acc, s2a, qxp[:, 2:16, 0:14],
                             start=False, stop=False)
            nc.tensor.matmul(acc, s2b, qxp[:, 1:15, 1:15],
                             start=False, stop=False)
            nc.tensor.matmul(acc, s1t, qxp[:, 2:16, 2:16],
                             start=False, stop=True)

            # ---- scale + bias, store ----
            osb = sb.tile([COUT, HW], F32, tag="osb")
            nc.vector.tensor_scalar(out=osb,
                                    in0=acc.rearrange("p h w -> p (h w)"),
                                    scalar1=alpha, scalar2=bt,
                                    op0=ALU.mult, op1=ALU.add)
            nc.sync.dma_start(out=out_d, in_=osb)

    nc.compile()
    return nc


_NC = None


def _get_nc():
    global _NC
    if _NC is None:
        _NC = build_nc()
    return _NC


def make_in_maps(x, weight, bias):
    x = np.ascontiguousarray(np.asarray(x, dtype=np.float32))
    w4 = np.asarray(weight, dtype=np.float32).reshape(COUT, CIN, 3, 3)
    w = np.ascontiguousarray(
        np.concatenate([w4[:, :, kh, kw] for kh, kw in TAP_ORDER], axis=1))
    b = np.ascontiguousarray(np.asarray(bias, dtype=np.float32).reshape(COUT, 1))
    xf = np.ascontiguousarray(x.reshape(128, 784))
    in_maps = []
    for c in range(N_CORES):
        img = x[c].reshape(64, HW)
        xi = np.ascontiguousarray(np.concatenate([img, img], axis=0))
        in_maps.append({"x_full": xf, "x_img": xi, "w_in": w, "b_in": b})
    return in_maps


def kernel(x, weight, bias, lut=None, gradient_lut=None, **_unused):
    nc = _get_nc()
    in_maps = make_in_maps(x, weight, bias)
    res = bass_utils.run_bass_kernel_spmd(nc, in_maps,
                                          core_ids=list(range(N_CORES)))
    out = np.stack([res.results[c]["out"] for c in range(N_CORES)], axis=0)
    return np.ascontiguousarray(out.reshape(B, COUT, H, W).astype(np.float32))
